# revision 1
# baseline (speedup 1.0000x reference)
"""Trainium2 Bass kernel for masked multi-head attention (B=4, N=1024, D=1024, H=16).

Sharding: 8 cores = 4 batches x 2 head-groups (tensor parallel over heads).
Each core computes QKV projection for its 8 heads, full attention, and a
partial output projection; the host sums the two partials per batch (+bout).

Returns (out, attn) matching the reference:
  out  [4, 1024, 1024] f32
  attn [4, 16, 1024, 1024] f32 (softmax probabilities)
"""

from contextlib import ExitStack

import numpy as np

import concourse.bass as bass
import concourse.tile as tile
from concourse import bacc, mybir
from concourse.bass_utils import run_bass_kernel_spmd
from concourse.masks import make_identity

B, N, D, H = 4, 1024, 1024, 16
DH = D // H  # 64
HL = H // 2  # heads per core = 8
SCALE = D ** -0.5
NEG = -30000.0  # additive mask; exp(x + NEG) underflows to exactly 0 in fp32

F32 = mybir.dt.float32
F32R = mybir.dt.float32r
AF = mybir.ActivationFunctionType
ALU = mybir.AluOpType

_NC = None


def _build():
    nc = bacc.Bacc("TRN2", target_bir_lowering=False, debug=False)

    xT = nc.dram_tensor("xT", [D, N], F32R, kind="ExternalInput").ap()
    w = nc.dram_tensor("w", [D, 3 * HL * DH], F32R, kind="ExternalInput").ap()
    bqk = nc.dram_tensor("bqk", [128, 8], F32, kind="ExternalInput").ap()
    bv = nc.dram_tensor("bv", [1, HL * DH], F32R, kind="ExternalInput").ap()
    wout = nc.dram_tensor("wout", [DH, HL, D], F32R, kind="ExternalInput").ap()
    amask = nc.dram_tensor("amask", [1, N], F32R, kind="ExternalInput").ap()
    onesd = nc.dram_tensor("onesd", [1, N], F32R, kind="ExternalInput").ap()
    rvt = nc.dram_tensor("rvt", [128, 8], F32, kind="ExternalInput").ap()
    cit = nc.dram_tensor("cit", [128, 8], F32, kind="ExternalInput").ap()

    attn_o = nc.dram_tensor("attn", [HL, N, N], F32, kind="ExternalOutput").ap()
    outp_o = nc.dram_tensor("outp", [N, D], F32, kind="ExternalOutput").ap()

    with tile.TileContext(nc) as tc, ExitStack() as ctx:
        smalls = ctx.enter_context(tc.tile_pool(name="smalls", bufs=1))
        persist = ctx.enter_context(tc.tile_pool(name="persist", bufs=1))
        ps_big = ctx.enter_context(tc.tile_pool(name="ps_big", bufs=2, space="PSUM"))
        ps_one = ctx.enter_context(tc.tile_pool(name="ps_one", bufs=2, space="PSUM"))
        ps_ovt = ctx.enter_context(tc.tile_pool(name="ps_ovt", bufs=1, space="PSUM"))

        ident = smalls.tile([128, 128], F32)
        make_identity(nc, ident)
        amask_sb = smalls.tile([1, N], F32R)
        nc.sync.dma_start(amask_sb[:], amask)
        ones_sb = smalls.tile([1, N], F32R)
        nc.sync.dma_start(ones_sb[:], onesd)
        rvt_sb = smalls.tile([128, 8], F32)
        nc.sync.dma_start(rvt_sb[:], rvt)
        cit_sb = smalls.tile([128, 8], F32)
        nc.sync.dma_start(cit_sb[:], cit)
        bqk_sb = smalls.tile([128, 8], F32)
        nc.sync.dma_start(bqk_sb[:], bqk)
        bv_sb = smalls.tile([1, HL * DH], F32R)
        nc.sync.dma_start(bv_sb[:], bv)

        # persistent activations
        qk_all = persist.tile([128, 8, N], F32R)   # q chunks 0..3, k chunks 4..7
        v_all = persist.tile([128, 8, HL * DH], F32R)  # v token-major, 8 token-chunks
        outT_all = persist.tile([DH, HL, N], F32R)
        wout_sb = persist.tile([DH, HL, D], F32R)
        nc.sync.dma_start(wout_sb[:], wout)

        # ---------------- Phase 1: QKV projection ----------------
        with tc.tile_pool(name="ph1", bufs=1) as ph1:
            xt_all = ph1.tile([128, 8, N], F32R)
            w_all = ph1.tile([128, 8, 3 * HL * DH], F32R)
            nc.sync.dma_start(
                xt_all[:], xT.rearrange("(dc p) n -> p dc n", p=128)
            )
            nc.sync.dma_start(
                w_all[:], w.rearrange("(dc p) f -> p dc f", p=128)
            )

            # q,k chunks: qkT[f, tok] = W[:, f].T @ xT
            for fc in range(8):
                ps = ps_big.tile([128, N], F32, tag="big")
                for hn in (0, 512):
                    for d in range(8):
                        nc.tensor.matmul(
                            ps[:, hn : hn + 512],
                            w_all[:, d, fc * 128 : (fc + 1) * 128],
                            xt_all[:, d, hn : hn + 512],
                            start=(d == 0),
                            stop=(d == 7),
                        )
                nc.scalar.activation(
                    qk_all[:, fc, :], ps[:],
                    AF.Identity, bias=bqk_sb[:, fc : fc + 1], scale=1.0,
                )

            # v token-major: v[tok, f] = xT[:, tok].T @ Wv  (+ bias via rank-1)
            for t in range(8):
                vps = ps_one.tile([128, 512], F32, tag="one")
                for d in range(8):
                    nc.tensor.matmul(
                        vps[:],
                        xt_all[:, d, t * 128 : (t + 1) * 128],
                        w_all[:, d, 2 * HL * DH : 3 * HL * DH],
                        start=(d == 0),
                        stop=False,
                    )
                nc.tensor.matmul(
                    vps[:], ones_sb[:, 0:128], bv_sb[:], start=False, stop=True
                )
                nc.scalar.copy(v_all[:, t, :], vps[:])

        # ---------------- Phase 2: attention per head ----------------
        aug = ctx.enter_context(tc.tile_pool(name="aug", bufs=2))
        pexp = ctx.enter_context(tc.tile_pool(name="pexp", bufs=2))
        pattn = ctx.enter_context(tc.tile_pool(name="pattn", bufs=3))
        ptall = ctx.enter_context(tc.tile_pool(name="ptall", bufs=1))
        tiny = ctx.enter_context(tc.tile_pool(name="tiny", bufs=4))

        for l in range(HL):
            fcq, po = l // 2, (l % 2) * 64
            qaug = aug.tile([65, N], F32R, tag="qaug")
            kaug = aug.tile([65, N], F32R, tag="kaug")
            nc.sync.dma_start(qaug[0:64, :], qk_all[po : po + 64, fcq, :])
            nc.sync.dma_start(qaug[64:65, :], ones_sb[:])
            nc.sync.dma_start(kaug[0:64, :], qk_all[po : po + 64, 4 + fcq, :])
            nc.sync.dma_start(kaug[64:65, :], amask_sb[:])

            pt_all = ptall.tile([128, 8, N], F32R, tag="pt")

            for ic in range(8):
                sps = ps_big.tile([128, N], F32, tag="big")
                for hn in (0, 512):
                    nc.tensor.matmul(
                        sps[:, hn : hn + 512],
                        qaug[:, ic * 128 : (ic + 1) * 128],
                        kaug[:, hn : hn + 512],
                        start=True,
                        stop=True,
                    )
                p_sb = pexp.tile([128, N], F32, tag="p")
                sums = tiny.tile([128, 1], F32, tag="sums")
                nc.scalar.activation(
                    p_sb[:], sps[:], AF.Exp, accum_out=sums[:]
                )
                rec = tiny.tile([128, 1], F32, tag="rec")
                nc.vector.reciprocal(rec[:], sums[:])
                s1 = tiny.tile([128, 1], F32, tag="s1")
                nc.vector.tensor_mul(s1[:], rec[:], rvt_sb[:, ic : ic + 1])
                attn_sb = pattn.tile([128, N], F32, tag="attn")
                nc.vector.tensor_scalar(
                    attn_sb[:], p_sb[:], s1[:], cit_sb[:, ic : ic + 1],
                    op0=ALU.mult, op1=ALU.add,
                )
                nc.sync.dma_start(
                    attn_o[l, ic * 128 : (ic + 1) * 128, :], attn_sb[:]
                )
                # transpose attn tile for the P@V matmul
                for jg in range(2):
                    ptps = ps_one.tile([128, 512], F32, tag="one")
                    for k in range(4):
                        jc = jg * 4 + k
                        nc.tensor.transpose(
                            ptps[:, k * 128 : (k + 1) * 128],
                            attn_sb[:, jc * 128 : (jc + 1) * 128],
                            ident[:],
                        )
                    nc.scalar.copy(
                        pt_all[:, jg * 4 : jg * 4 + 4, ic * 128 : (ic + 1) * 128],
                        ptps[:].rearrange("p (g n) -> p g n", g=4),
                    )

            # P@V: outT[d, i] += v[jc]^T-slice . PT[jc]
            ovt = ps_ovt.tile([DH, N], F32, tag="ovt")
            for hn in (0, 512):
                for jc in range(8):
                    nc.tensor.matmul(
                        ovt[:, hn : hn + 512],
                        v_all[:, jc, l * DH : (l + 1) * DH],
                        pt_all[:, jc, hn : hn + 512],
                        start=(jc == 0),
                        stop=(jc == 7),
                    )
            nc.scalar.copy(outT_all[:, l, :], ovt[:])

        # ---------------- Phase 3: output projection (partial) ----------------
        with tc.tile_pool(name="pout", bufs=2) as pout:
            for ic in range(8):
                ops = ps_big.tile([128, D], F32, tag="big")
                for hn in (0, 512):
                    for l in range(HL):
                        nc.tensor.matmul(
                            ops[:, hn : hn + 512],
                            outT_all[:, l, ic * 128 : (ic + 1) * 128],
                            wout_sb[:, l, hn : hn + 512],
                            start=(l == 0),
                            stop=(l == HL - 1),
                        )
                osb = pout.tile([128, D], F32, tag="o")
                nc.scalar.copy(osb[:], ops[:])
                nc.sync.dma_start(outp_o[ic * 128 : (ic + 1) * 128, :], osb[:])

    nc.compile()
    return nc


def _get_nc():
    global _NC
    if _NC is None:
        _NC = _build()
    return _NC


def _prep_core(x, mask, Wqkv, bqkv, Wout, c):
    b, half = c // 2, c % 2
    s, e = half * 512, (half + 1) * 512

    xT = np.ascontiguousarray(x[b].T)

    Wq = Wqkv[:, s:e] * np.float32(SCALE)
    Wk = Wqkv[:, D + s : D + e]
    Wv = Wqkv[:, 2 * D + s : 2 * D + e]
    w = np.ascontiguousarray(np.concatenate([Wq, Wk, Wv], axis=1))

    bq = bqkv[s:e] * np.float32(SCALE)
    bk = bqkv[D + s : D + e]
    bqk = np.ascontiguousarray(
        np.concatenate([bq, bk]).reshape(8, 128).T
    )
    bv = np.ascontiguousarray(bqkv[2 * D + s : 2 * D + e].reshape(1, 512))

    # wout[dh, l, e] = Wout[half*512 + l*64 + dh, e]
    wout = np.ascontiguousarray(
        Wout[s:e, :].reshape(HL, DH, D).transpose(1, 0, 2)
    )

    m = np.concatenate([[True], mask[b]])  # [N]
    amask = np.where(m, 0.0, NEG).astype(np.float32).reshape(1, N)
    rv = m.astype(np.float32)
    rvt = np.ascontiguousarray(rv.reshape(8, 128).T)
    cit = np.ascontiguousarray(((1.0 - rv) / N).astype(np.float32).reshape(8, 128).T)

    return {
        "xT": xT.astype(np.float32),
        "w": w.astype(np.float32),
        "bqk": bqk.astype(np.float32),
        "bv": bv.astype(np.float32),
        "wout": wout.astype(np.float32),
        "amask": amask,
        "onesd": np.ones((1, N), np.float32),
        "rvt": rvt,
        "cit": cit,
    }


def kernel(x, mask, Wqkv, bqkv, Wout, bout):
    x = np.asarray(x, dtype=np.float32)
    mask = np.asarray(mask)
    Wqkv = np.asarray(Wqkv, dtype=np.float32)
    bqkv = np.asarray(bqkv, dtype=np.float32)
    Wout = np.asarray(Wout, dtype=np.float32)
    bout = np.asarray(bout, dtype=np.float32)

    nc = _get_nc()
    in_maps = [_prep_core(x, mask, Wqkv, bqkv, Wout, c) for c in range(8)]
    res = run_bass_kernel_spmd(nc, in_maps, core_ids=list(range(8)))

    attn = np.empty((B, H, N, N), np.float32)
    out = np.empty((B, N, D), np.float32)
    for c in range(8):
        b, half = c // 2, c % 2
        attn[b, half * HL : (half + 1) * HL] = res.results[c]["attn"]
    for b in range(B):
        out[b] = res.results[2 * b]["outp"] + res.results[2 * b + 1]["outp"] + bout
    return (out, attn)
